# revision 3
# baseline (speedup 1.0000x reference)
"""Multi-head causal self-attention (B=2, S=2048, D=1024, H=16) on 8 TRN2
NeuronCores via Bass/Tile.

HW-calibrated optimizations over the original version:
- Score matmuls contract K=128 instead of K=64 (HW runs K<=64 matmuls at
  ~2x cycles/col): per-head zero-padded Q^T tiles against the packed
  two-head K^T e-block tile (zeros annihilate the other head's term).
- Both heads' scores land in one 2-bank PSUM tile; a single wide Exp per
  item halves ACT instruction count.
- Diagonal mask multiplies run on the otherwise idle Pool engine.
- The softmax-reciprocal broadcast matmul contracts K=128 via zero-padded
  selector/reciprocal tiles instead of K=1.
- The score->exp->AV pipeline carries across q-chunk boundaries (one global
  pend queue), eliminating per-chunk drain bubbles; lag=7 ~ the measured
  optimum (cross-engine handoff latency is ~250-300 ns, so deep pipelining
  matters more than engine busy-time).

Sharding: core c -> (batch b = c // 4, head-group g = c % 4). Each core
computes q/k/v projections for its 4 heads (256 of 1024 projection cols),
causal flash attention for those heads, and a partial output projection
(row-parallel over the head dim). Host sums the 4 partials per batch.

Device layouts (all transposed so the contraction dim sits on partitions):
  xT   [D, S]   : x[b].T, host-transposed, bf16
  Q^T/K^T [e, S]: head dim on partitions, bf16
  V    [k, e+1] : natural, with a ones column per head; the ones column turns
                  the AV^T matmul into (unnormalized AV^T, softmax denom) rows
  A^T  [e, S]   : produced directly by AV^T matmul, consumed as moving
                  operand of the output projection -> zero on-chip transposes
  outT [D, S]   : transposed partial output (bf16), host sums + transposes

All matmul operands are bf16 (full PE rate at any moving width, half the
DMA/SBUF footprint of fp32; rel err ~4e-3 vs the f32 reference). PSUM stays
f32. Engine split: PE all matmuls (incl. the reciprocal partition-broadcast
as a K=1 outer product), ACT only Exp, DVE masking/normalize/projection
evictions, Pool V/output evictions, SP issues all input DMAs (HWDGE only,
chunked so compute starts as soon as the first pieces land), DVE queue
issues output DMAs.

Schedule: software-pipelined at j-block granularity. Scores/AV for the four
heads are interleaved within each k-block group so PE never sits behind a
single exp->mask->AV dependency chain, and projection matmul groups for
chunk c+1 plus output-projection groups for chunk c-1 are spread evenly
between the j-groups of chunk c's attention.

Scores are computed as S^T[k, q] = (K^T_blk)^T @ Q^T so softmax reduces over
the partition dim (folded into the AV matmul via the ones column). exp()
needs no max-subtraction: scores are O(1) here.
"""

from contextlib import ExitStack

import numpy as np
import ml_dtypes

import concourse.bass as bass
import concourse.mybir as mybir
import concourse.tile as tile
from concourse.bass_utils import run_bass_kernel_spmd

# Problem constants (hardcoded per harness contract).
B, S, D, NH, DH = 2, 2048, 1024, 16, 64
N_CORES = 8
GROUPS = 4                 # head-groups; cores per batch
HPC = NH // GROUPS         # heads per core = 4
E = HPC * DH               # per-core projection width = 256
P = 128                    # SBUF partitions
SC = 512                   # moving-operand chunk (q chunk)
ND = D // P                # 8 d-chunks
NEB = E // P               # 2 e-blocks per core
NQ = S // SC               # 4 q chunks
NKB = S // P               # 16 k blocks
SCALE = DH ** -0.5

F32 = mybir.dt.float32
BF = mybir.dt.bfloat16


def _split_multiwait(nc, max_waits=1):
    """This toolchain's walrus codegen accepts at most one sync-wait per
    instruction ("Too many sync wait commands"). Tile emits multi-wait
    instructions (notably the kernel-tail Drain). Keep the last wait (+ all
    updates) on the original instruction and hoist earlier waits onto
    single-wait Drains inserted before it on the same engine."""
    for f in nc.m.functions:
        for bb in f.blocks:
            new = []
            changed = False
            for inst in bb.instructions:
                si = inst.sync_info
                waits = list(si.on_wait) if si is not None and si.on_wait else []
                if len(waits) > max_waits:
                    for j, w in enumerate(waits[:-max_waits]):
                        d = mybir.InstDrain(name=f"{inst.name}-sw{j}", ins=[], outs=[])
                        d.engine = inst.engine
                        d.sync_info = mybir.SyncInfo(on_wait=[w], on_update=[])
                        new.append(d)
                    inst.sync_info = mybir.SyncInfo(
                        on_wait=waits[-max_waits:],
                        on_update=list(si.on_update) if si.on_update else [],
                    )
                    changed = True
                new.append(inst)
            if changed:
                bb.instructions = new


def build_nc(repeat=1, lag=5, ablate=None):
    """repeat>1 wraps the whole body in a hardware For_i loop — used only by
    the benchmark to amortize dispatch overhead out of wall-clock timing.
    lag = how many j-blocks the score/exp pipeline runs ahead of AV.
    ablate: timing-forensics only — 'nodma' skips input DMAs (outputs become
    garbage; isolates DMA cost on HW)."""
    nc = bass.Bass("TRN2", target_bir_lowering=False, debug=False,
                   num_devices=N_CORES)

    xT = nc.dram_tensor("xT", [D, S], BF, kind="ExternalInput")
    wqT = nc.dram_tensor("wqT", [D, E], BF, kind="ExternalInput")
    wkT = nc.dram_tensor("wkT", [D, E], BF, kind="ExternalInput")
    wvT = nc.dram_tensor("wvT", [D, E], BF, kind="ExternalInput")
    woT = nc.dram_tensor("woT", [E, D], BF, kind="ExternalInput")
    bqk = nc.dram_tensor("bqk", [E, 2], F32, kind="ExternalInput")
    outT = nc.dram_tensor("outT", [D, S], BF, kind="ExternalOutput")

    AF = mybir.ActivationFunctionType
    with tile.TileContext(nc) as tc:
        with ExitStack() as ctx:
            if repeat > 1:
                ctx.enter_context(tc.For_i(0, repeat, 1))
            const = ctx.enter_context(tc.tile_pool(name="const", bufs=1))

            # ---- persistent SBUF tensors ----
            # x per q-chunk (one DMA each, pipelined); weights one DMA each
            x_sbs = [const.tile([P, ND, SC], BF, tag=f"x{c}", name=f"x{c}")
                     for c in range(NQ)]
            wq_sb = const.tile([P, ND, E], BF, tag="wq", name="wq")
            wk_sb = const.tile([P, ND, E], BF, tag="wk", name="wk")
            wv_sb = const.tile([P, ND, E], BF, tag="wv", name="wv")
            wo_sb = const.tile([P, NEB, D], BF, tag="wo", name="wo")
            bqk_sb = const.tile([P, NEB, 2], F32, tag="bqk", name="bqk")
            # Q^T/K^T per (e-block, q-chunk); V per 512-wide k-chunk
            qts = [[const.tile([P, SC], BF, tag=f"qt{h}{c}", name=f"qt{h}{c}") for c in range(NQ)]
                   for h in range(HPC)]
            kts = [[const.tile([P, SC], BF, tag=f"kt{e}{c}", name=f"kt{e}{c}") for c in range(NQ)]
                   for e in range(NEB)]
            v_sbs = [const.tile([P, NQ, HPC * (DH + 1)], BF, tag=f"v{i}", name=f"v{i}")
                     for i in range(NQ)]
            at_sbs = [[const.tile([P, SC], BF, tag=f"at{i}{f}", name=f"at{i}{f}")
                       for f in range(NEB)] for i in range(NQ)]
            mk_sb = const.tile([P, NQ, SC], BF, tag="mk", name="mk")
            # packed mask for the fused (m=1, m=3) diagonal pair: columns
            # 0:384 hold m1's active window (== m0's mask restricted there),
            # columns 384:512 hold m3's
            mkp_sb = const.tile([P, SC], BF, tag="mkp", name="mkp")
            # sel[hh][0, m] = 1 iff head hh of the pair owns at-row m (K=1
            # outer products broadcast both heads' softmax reciprocals into
            # one [128, SC] tile)
            # sel2[hh]: row 0 = head hh's partition-range selector, rows
            # 1..127 zero; rc2 row 0 = reciprocal row, rows 1..127 zero =>
            # the broadcast matmul contracts K=128 at full rate
            sel_sbs = [const.tile([P, P], BF, tag=f"sel{hh}", name=f"sel{hh}")
                       for hh in range(2)]
            rc2 = const.tile([P, 2, SC], BF, tag="rc2", name="rc2")
            nc.vector.memset(rc2[:], 0.0)

            # ---- input DMAs: one per tensor / half-x-chunk, compute-ordered
            xTr = xT.rearrange("(n p) s -> p n s", p=P)

            def dma_x(c, part, of):
                nd = ND // of
                nc.sync.dma_start(
                    x_sbs[c][:, part * nd:(part + 1) * nd, :],
                    xTr[:, part * nd:(part + 1) * nd, c * SC:(c + 1) * SC])
            if ablate != "nodma":
                wkTr = wkT.rearrange("(n p) e -> p n e", p=P)
                nd2 = ND // 2
                # x chunk 0 in quarters interleaved with wk halves so the
                # first K-projection matmuls unblock as early as possible
                nc.sync.dma_start(wk_sb[:, :nd2, :], wkTr[:, :nd2, :])
                dma_x(0, 0, 4)
                dma_x(0, 1, 4)
                nc.sync.dma_start(wk_sb[:, nd2:, :], wkTr[:, nd2:, :])
                dma_x(0, 2, 4)
                dma_x(0, 3, 4)
                nc.sync.dma_start(bqk_sb[:], bqk.rearrange("(n p) two -> p n two", p=P))
                nc.sync.dma_start(wq_sb[:], wqT.rearrange("(n p) e -> p n e", p=P))
                nc.sync.dma_start(wv_sb[:], wvT.rearrange("(n p) e -> p n e", p=P))
                for c in range(1, NQ):
                    dma_x(c, 0, 2)
                    dma_x(c, 1, 2)
                nc.sync.dma_start(wo_sb[:], woT.rearrange("(n p) d -> p n d", p=P))
            else:
                nc.vector.memset(bqk_sb[:], 0.0)

            # constants: head-pair selector + multiplicative causal masks
            tmp = ctx.enter_context(tc.tile_pool(name="tmp", bufs=1))
            one_f32 = tmp.tile([P, 1], F32, tag="onef", name="onef")
            nc.vector.memset(one_f32[:], 1.0)
            for hh in range(2):
                nc.vector.memset(sel_sbs[hh][:], 0.0)
                nc.vector.memset(sel_sbs[hh][0:1, hh * DH:(hh + 1) * DH], 1.0)
            # mk[m][kk, qq] = 1.0 if kk + 128*m <= qq else 0.0
            mkf_sb = tmp.tile([P, NQ, SC], F32, tag="mkf", name="mkf")
            for m in range(NQ):
                nc.gpsimd.memset(mkf_sb[:, m, :], 1.0)
                nc.gpsimd.affine_select(
                    out=mkf_sb[:, m, :], in_=mkf_sb[:, m, :],
                    compare_op=mybir.AluOpType.is_ge, fill=0.0,
                    base=-(P * m), pattern=[[1, SC]], channel_multiplier=-1,
                )
            nc.vector.tensor_copy(mk_sb[:], mkf_sb[:])
            nc.vector.tensor_copy(mkp_sb[:, 0:384], mkf_sb[:, 0, 0:384])
            nc.vector.tensor_copy(mkp_sb[:, 384:], mkf_sb[:, 3, 384:])
            for cc in range(NQ):
                nc.vector.tensor_copy(
                    v_sbs[cc][:, :, DH::DH + 1],
                    one_f32[:, :, None].broadcast_to([P, NQ, HPC]))
            for h in range(HPC):
                hh = h % 2
                for cc in range(NQ):
                    nc.gpsimd.memset(qts[h][cc][(1 - hh) * DH:(2 - hh) * DH, :], 0.0)

            # PSUM: pproj 1 + psc 4 + pav 2 + pmx 1 = 8 banks.
            pproj = ctx.enter_context(tc.tile_pool(name="pproj", bufs=1, space="PSUM"))
            psc = ctx.enter_context(tc.tile_pool(name="psc", bufs=2, space="PSUM"))
            pav = ctx.enter_context(tc.tile_pool(name="pav", bufs=1, space="PSUM"))
            pmx = ctx.enter_context(tc.tile_pool(name="pmx", bufs=1, space="PSUM"))
            ptp = ctx.enter_context(tc.tile_pool(name="ptp", bufs=min(2 * lag + 6, 28)))
            rcp = ctx.enter_context(tc.tile_pool(name="rcp", bufs=2))
            obp = ctx.enter_context(tc.tile_pool(name="obp", bufs=2))

            # ---- work-item generators (each item ~1 PSUM group on PE) ----
            def proj_pool(pool):
                # allocate a [P, SC] f32 group tile from the given PSUM pool
                # (tag must match the pool's existing tag to avoid growing it)
                tag = {id(pproj): "pj", id(pmx): "mx", id(psc): "sc"}[id(pool)]
                return pool.tile([P, SC], F32, tag=tag, name="pjx")

            def proj_qk_group(w_sb, bcol, o_tiles, c, eb, pool=None):
                ps = proj_pool(pool or pproj)
                for di in range(ND):
                    nc.tensor.matmul(
                        ps[:],
                        lhsT=w_sb[:, di, eb * P:(eb + 1) * P],
                        rhs=x_sbs[c][:, di, :],
                        start=(di == 0), stop=(di == ND - 1),
                    )
                if bcol == 1:   # K^T: packed two-head tile
                    nc.vector.tensor_scalar_add(
                        out=o_tiles[eb][c][:], in0=ps[:],
                        scalar1=bqk_sb[:, eb, bcol:bcol + 1])
                else:           # Q^T: per-head zero-padded tiles
                    for hh in range(2):
                        rows = slice(hh * DH, (hh + 1) * DH)
                        nc.vector.tensor_scalar_add(
                            out=o_tiles[2 * eb + hh][c][rows, :],
                            in0=ps[rows, :],
                            scalar1=bqk_sb[rows, eb, 0:1])

            def proj_v_group(c, kk, pool=None):
                ps = proj_pool(pool or pproj)
                for di in range(ND):
                    nc.tensor.matmul(
                        ps[:, :E],
                        lhsT=x_sbs[c][:, di, kk * P:(kk + 1) * P],
                        rhs=wv_sb[:, di, :],
                        start=(di == 0), stop=(di == ND - 1),
                    )
                dst = v_sbs[c][:, kk, :].rearrange(
                    "p (h e) -> p h e", h=HPC)[:, :, :DH]
                nc.vector.tensor_copy(
                    dst, ps[:, :E].rearrange("p (h e) -> p h e", h=HPC))

            def proj_items(c, pools=None):
                # pools: optional PSUM rotation for the startup prologue,
                # when the attention pools are still idle
                pl = (lambda i: pools[i % len(pools)]) if pools else \
                    (lambda i: None)
                items = []
                for eb in range(NEB):
                    items.append(lambda eb=eb, i=len(items):
                                 proj_qk_group(wk_sb, 1, kts, c, eb, pl(i)))
                for eb in range(NEB):
                    items.append(lambda eb=eb, i=len(items):
                                 proj_qk_group(wq_sb, 0, qts, c, eb, pl(i)))
                for kk in range(NQ):
                    items.append(lambda kk=kk, i=len(items):
                                 proj_v_group(c, kk, pl(i)))
                return items

            outTr = outT.rearrange("(n p) s -> p n s", p=P)
            ob_tiles = {}

            def outproj_group(c, eb, pool=None):
                if eb == 0:
                    ob_tiles[c] = obp.tile([P, ND, SC], BF, tag="ob", name="ob")
                po = proj_pool(pool or pmx)
                for ft in range(NEB):
                    nc.tensor.matmul(
                        po[:],
                        lhsT=wo_sb[:, ft, eb * P:(eb + 1) * P],
                        rhs=at_sbs[c][ft][:],
                        start=(ft == 0), stop=(ft == NEB - 1),
                    )
                tail = c == NQ - 1
                if tail and eb == ND - 2:
                    # second-to-last eviction on ACT (idle at the tail;
                    # Copy shares Exp's activation table) so the last two
                    # evictions run on different engines in parallel
                    nc.scalar.activation(ob_tiles[c][:, eb, :], po[:],
                                         AF.Copy)
                else:
                    nc.vector.tensor_copy(ob_tiles[c][:, eb, :], po[:])
                # store in halves; quarters for the last chunk (singles for
                # its final two blocks) so the DMA on the kernel critical
                # tail is as small as possible
                grp = 1 if (tail and eb >= ND - 2) else \
                    (2 if tail else ND // 2)
                if (eb + 1) % grp == 0:
                    lo = eb + 1 - grp
                    nc.sync.dma_start(
                        outTr[:, lo:eb + 1, c * SC:(c + 1) * SC],
                        ob_tiles[c][:, lo:eb + 1, :])

            def outproj_items(c, pools=(None,)):
                # pools: PSUM bank rotation so group k+1's matmuls don't
                # wait on group k's eviction (pass idle pools only)
                return [lambda eb=eb: outproj_group(c, eb,
                                                    pools[eb % len(pools)])
                        for eb in range(ND)]

            # ---- attention for chunk c, with fill items interleaved ----
            # Heads run in pairs (one pass over all k-blocks per pair): the
            # pair's two score matmuls land in one 2-bank PSUM tile so a
            # single wide Exp covers both heads (halves ACT instruction
            # overhead), and only 2 AV accumulator banks are live at a time.
            pend = []

            def flush_one():
                c0, hp, item, pt, first, last, av_fns = pend.pop(0)
                av_fns(c0, hp, item, pt, first, last)

            def attention_chunk(c, fill):
                nj = NQ * (c + 1)
                filled = 0

                # Work items: plain k-blocks, then the diagonal emitted
                # as m0, m2, and a fused (m1, m3) pair that shares one
                # PSUM bank / Exp / mask per head (the pair's 384+128
                # active columns exactly fill a bank). Each item is a
                # list of (j, q0, col0) score segments.
                items = [[(j, 0, 0)] for j in range(NQ * c)]
                items.append([(NQ * c, 0, 0)])                    # m0
                items.append([(NQ * c + 2, 2 * P, 2 * P)])        # m2
                items.append([(NQ * c + 1, P, 0),                 # m1
                              (NQ * c + 3, 3 * P, 3 * P)])        # m3
                ni = len(items)
                nsteps = 2 * ni

                def bankcols(item, q0, col0):
                    # bank columns backing q-range [q0:SC] of a segment:
                    # packed segments sit at [col0 : col0 + width]
                    if len(item) > 1:
                        return slice(col0, col0 + (SC - q0))
                    return slice(q0, SC)

                def banklo(item):
                    return 0 if len(item) > 1 else item[0][1]

                def scores(hp, item):
                    # both heads' scores land in one 2-bank psum tile so a
                    # single wide Exp covers the pair. Contraction is K=128:
                    # the packed K^T e-block against the per-head zero-padded
                    # Q^T (K<=64 matmuls cost ~2x cycles/col on HW).
                    pt = ptp.tile([P, 2, SC], BF, tag="pt", name="pt")
                    packed = len(item) > 1
                    lo = banklo(item)
                    ps = psc.tile([P, 2, SC], F32, tag="sc", name="sc")
                    for j, q0, col0 in item:
                        for hh in range(2):  # hh inner: shared kts stationary
                            nc.tensor.matmul(
                                ps[:, hh, bankcols(item, q0, col0)],
                                lhsT=kts[hp][j // NQ][:, (j % NQ) * P:(j % NQ + 1) * P],
                                rhs=qts[2 * hp + hh][c][:, q0:],
                                start=True, stop=True,
                            )
                    nc.scalar.activation(pt[:, :, lo:], ps[:, :, lo:],
                                         AF.Exp, scale=SCALE)
                    m = item[0][0] - NQ * c
                    if m >= 0:   # diagonal: mask (packed pair has mkp)
                        mk = mkp_sb[:, lo:] if packed else mk_sb[:, m, lo:]
                        nc.gpsimd.tensor_mul(
                            pt[:, :, lo:], pt[:, :, lo:],
                            mk[:, None, :].broadcast_to([P, 2, SC - lo]))
                    return pt

                av_tiles = {}

                def avs(hp, item, pt, first, last):
                    if first:
                        av_tiles[hp] = [pav.tile([DH + 1, SC], F32,
                                                 tag=f"av{i}", name=f"av{i}")
                                        for i in range(2)]
                    for hh in range(2):
                        h = 2 * hp + hh
                        for idx, (j, q0, col0) in enumerate(item):
                            nc.tensor.matmul(
                                av_tiles[hp][hh][:, q0:],
                                lhsT=v_sbs[j // NQ][:, j % NQ,
                                                    h * (DH + 1):(h + 1) * (DH + 1)],
                                rhs=pt[:, hh, bankcols(item, q0, col0)],
                                start=first and idx == 0,
                                stop=last and idx == len(item) - 1,
                            )
                    if last:
                        normalize(hp)

                def normalize(hp):
                    # A^T[f, q] = av[f, q] * (1 / denom[q]); both heads'
                    # reciprocal rows are broadcast into one [128, SC] tile
                    # via two accumulating K=1 outer products with the
                    # selector rows (DVE writes must start at partition 0,
                    # so the reciprocals live in separate single-row tiles).
                    rb_ps = pmx.tile([P, SC], F32, tag="mx", name="mx")
                    with nc.allow_low_precision(
                            reason="bf16 rounding of softmax recip is benign"):
                        for hh in range(2):
                            nc.vector.reciprocal(
                                rc2[0:1, hh, :], av_tiles[hp][hh][DH:DH + 1, :])
                            nc.tensor.matmul(
                                rb_ps[:], lhsT=sel_sbs[hh][:, :],
                                rhs=rc2[:, hh, :],
                                start=(hh == 0), stop=(hh == 1))
                    rcb = rcp.tile([P, SC], BF, tag="rcb", name="rcb")
                    nc.vector.tensor_copy(rcb[:], rb_ps[:])
                    with nc.allow_low_precision(
                            reason="bf16 attention weights are benign"):
                        for hh in range(2):
                            nc.vector.tensor_mul(
                                at_sbs[c][hp][hh * DH:(hh + 1) * DH, :],
                                av_tiles[hp][hh][0:DH, :],
                                rcb[hh * DH:(hh + 1) * DH, :])

                # single merged stream over both head-pair passes AND across
                # chunks: the pend queue carries over so chunk c+1's scores
                # overlap chunk c's AV drain / normalize (no boundary bubble)
                def av_fns(c0, hp, item, pt, first, last):
                    avs(hp, item, pt, first, last)

                stream = [(hp, i) for hp in range(2) for i in range(ni)]
                for step, (hp, i) in enumerate(stream):
                    pend.append((c, hp, items[i], scores(hp, items[i]),
                                 i == 0, i == ni - 1, av_fns))
                    if len(pend) > lag:
                        flush_one()
                    want = (step + 1) * len(fill) // nsteps
                    while filled < want:
                        fill[filled]()
                        filled += 1

            # ---- software-pipelined schedule ----
            # proj(0) runs standalone (it is the DMA-paced startup); then
            # attention(c) hides proj(c+1) and outproj(c-1); outproj(3) tails.
            for item in proj_items(0, pools=[pproj, pmx, psc, psc]):
                item()
            rot2 = (pmx, pproj)
            fills = {0: proj_items(1),
                     1: proj_items(2),
                     2: proj_items(3) + outproj_items(0),
                     3: outproj_items(1, rot2) + outproj_items(2, rot2)}
            for c in range(NQ):
                attention_chunk(c, fills[c])
            while pend:
                flush_one()
            # pure tail: psc's banks are free too — 3-deep rotation hides
            # the PSUM-eviction latency between groups
            for item in outproj_items(NQ - 1, (pmx, pproj, psc)):
                item()

    _split_multiwait(nc)
    return nc


_NC_CACHE = None


def _shard_inputs(inputs):
    bf = ml_dtypes.bfloat16
    x = np.asarray(inputs["x"], np.float32)
    Wq = np.asarray(inputs["Wq"], np.float32)
    Wk = np.asarray(inputs["Wk"], np.float32)
    Wv = np.asarray(inputs["Wv"], np.float32)
    Wo = np.asarray(inputs["Wo"], np.float32)
    bq = np.asarray(inputs["bq"], np.float32)
    bk = np.asarray(inputs["bk"], np.float32)

    xTs = [np.ascontiguousarray(x[b].T).astype(bf) for b in range(B)]
    in_maps = []
    for c in range(N_CORES):
        b, g = divmod(c, GROUPS)
        rows = slice(g * E, (g + 1) * E)
        in_maps.append({
            "xT": xTs[b],
            "wqT": np.ascontiguousarray(Wq[rows].T).astype(bf),
            "wkT": np.ascontiguousarray(Wk[rows].T).astype(bf),
            "wvT": np.ascontiguousarray(Wv[rows].T).astype(bf),
            "woT": np.ascontiguousarray(Wo[:, rows].T).astype(bf),
            "bqk": np.ascontiguousarray(
                np.stack([bq[rows], bk[rows]], axis=1)),
        })
    return in_maps


def kernel(**inputs):
    global _NC_CACHE
    if _NC_CACHE is None:
        _NC_CACHE = build_nc()
    nc = _NC_CACHE

    # The mask input is causal (tril ones) by construction; the kernel
    # hardcodes causal structure.
    in_maps = _shard_inputs(inputs)
    res = run_bass_kernel_spmd(nc, in_maps, list(range(N_CORES)))

    Wo = np.asarray(inputs["Wo"], np.float32)
    bv = np.asarray(inputs["bv"], np.float32)
    bo = np.asarray(inputs["bo"], np.float32)
    out = np.zeros((B, S, D), np.float32)
    for c in range(N_CORES):
        b = c // GROUPS
        out[b] += res.results[c]["outT"].astype(np.float32).T
    # bv enters only additively after softmax (rows of P sum to 1):
    # out += Wo @ bv; plus the output bias bo.
    out += (Wo @ bv + bo)[None, None, :]
    return out



# revision 6
# speedup vs baseline: 1.0552x; 1.0552x over previous
"""Multi-head causal self-attention (B=2, S=2048, D=1024, H=16) on 8 TRN2
NeuronCores via Bass/Tile.

HW-calibrated optimizations over the original version:
- Score matmuls contract K=128 instead of K=64 (HW runs K<=64 matmuls at
  ~2x cycles/col): per-head zero-padded Q^T tiles against the packed
  two-head K^T e-block tile (zeros annihilate the other head's term).
- Both heads' scores land in one 2-bank PSUM tile; a single wide Exp per
  item halves ACT instruction count.
- Diagonal mask multiplies run on the otherwise idle Pool engine.
- The softmax-reciprocal broadcast matmul contracts K=128 via zero-padded
  selector/reciprocal tiles instead of K=1.
- The score->exp->AV pipeline carries across q-chunk boundaries (one global
  pend queue), eliminating per-chunk drain bubbles (cross-engine handoff
  latency is ~250-300 ns, so deep pipelining matters more than busy-time).

Sharding: core c -> (batch b = c // 4, head-group g = c % 4). Each core
computes q/k/v projections for its 4 heads (256 of 1024 projection cols),
causal flash attention for those heads, and a partial output projection
(row-parallel over the head dim). Host sums the 4 partials per batch.

Device layouts (all transposed so the contraction dim sits on partitions):
  xT   [D, S]   : x[b].T, host-transposed, bf16
  Q^T/K^T [e, S]: head dim on partitions, bf16
  V    [k, e+1] : natural, with a ones column per head; the ones column turns
                  the AV^T matmul into (unnormalized AV^T, softmax denom) rows
  A^T  [e, S]   : produced directly by AV^T matmul, consumed as moving
                  operand of the output projection -> zero on-chip transposes
  outT [D, S]   : transposed partial output (bf16), host sums + transposes

All matmul operands are bf16 (full PE rate at any moving width, half the
DMA/SBUF footprint of fp32; rel err ~4e-3 vs the f32 reference). PSUM stays
f32. Engine split: PE all matmuls (incl. the reciprocal partition-broadcast
as a K=1 outer product), ACT only Exp, DVE masking/normalize/projection
evictions, Pool V/output evictions, SP issues all input DMAs (HWDGE only,
chunked so compute starts as soon as the first pieces land), DVE queue
issues output DMAs.

Schedule: software-pipelined at j-block granularity. Scores/AV for the four
heads are interleaved within each k-block group so PE never sits behind a
single exp->mask->AV dependency chain, and projection matmul groups for
chunk c+1 plus output-projection groups for chunk c-1 are spread evenly
between the j-groups of chunk c's attention.

Scores are computed as S^T[k, q] = (K^T_blk)^T @ Q^T so softmax reduces over
the partition dim (folded into the AV matmul via the ones column). exp()
needs no max-subtraction: scores are O(1) here.
"""

from contextlib import ExitStack

import numpy as np
import ml_dtypes

import concourse.bass as bass
import concourse.mybir as mybir
import concourse.tile as tile
from concourse.bass_utils import run_bass_kernel_spmd

# Problem constants (hardcoded per harness contract).
B, S, D, NH, DH = 2, 2048, 1024, 16, 64
N_CORES = 8
GROUPS = 4                 # head-groups; cores per batch
HPC = NH // GROUPS         # heads per core = 4
E = HPC * DH               # per-core projection width = 256
P = 128                    # SBUF partitions
SC = 512                   # moving-operand chunk (q chunk)
ND = D // P                # 8 d-chunks
NEB = E // P               # 2 e-blocks per core
NQ = S // SC               # 4 q chunks
NKB = S // P               # 16 k blocks
SCALE = DH ** -0.5

F32 = mybir.dt.float32
BF = mybir.dt.bfloat16


def _split_multiwait(nc, max_waits=1):
    """This toolchain's walrus codegen accepts at most one sync-wait per
    instruction ("Too many sync wait commands"). Tile emits multi-wait
    instructions (notably the kernel-tail Drain). Keep the last wait (+ all
    updates) on the original instruction and hoist earlier waits onto
    single-wait Drains inserted before it on the same engine."""
    for f in nc.m.functions:
        for bb in f.blocks:
            new = []
            changed = False
            for inst in bb.instructions:
                si = inst.sync_info
                waits = list(si.on_wait) if si is not None and si.on_wait else []
                if len(waits) > max_waits:
                    for j, w in enumerate(waits[:-max_waits]):
                        d = mybir.InstDrain(name=f"{inst.name}-sw{j}", ins=[], outs=[])
                        d.engine = inst.engine
                        d.sync_info = mybir.SyncInfo(on_wait=[w], on_update=[])
                        new.append(d)
                    inst.sync_info = mybir.SyncInfo(
                        on_wait=waits[-max_waits:],
                        on_update=list(si.on_update) if si.on_update else [],
                    )
                    changed = True
                new.append(inst)
            if changed:
                bb.instructions = new


def build_nc(repeat=1, lag=5, ablate=None):
    """repeat>1 wraps the whole body in a hardware For_i loop — used only by
    the benchmark to amortize dispatch overhead out of wall-clock timing.
    lag = how many j-blocks the score/exp pipeline runs ahead of AV.
    ablate: timing-forensics only — 'nodma' skips input DMAs (outputs become
    garbage; isolates DMA cost on HW)."""
    nc = bass.Bass("TRN2", target_bir_lowering=False, debug=False,
                   num_devices=N_CORES)

    xT = nc.dram_tensor("xT", [D, S], BF, kind="ExternalInput")
    wqT = nc.dram_tensor("wqT", [D, E], BF, kind="ExternalInput")
    wkT = nc.dram_tensor("wkT", [D, E], BF, kind="ExternalInput")
    wvT = nc.dram_tensor("wvT", [D, E], BF, kind="ExternalInput")
    woT = nc.dram_tensor("woT", [E, D], BF, kind="ExternalInput")
    bqk = nc.dram_tensor("bqk", [E, 2], F32, kind="ExternalInput")
    outT = nc.dram_tensor("outT", [D, S], BF, kind="ExternalOutput")

    AF = mybir.ActivationFunctionType
    with tile.TileContext(nc) as tc:
        with ExitStack() as ctx:
            if repeat > 1:
                ctx.enter_context(tc.For_i(0, repeat, 1))
            const = ctx.enter_context(tc.tile_pool(name="const", bufs=1))

            # ---- persistent SBUF tensors ----
            # x per q-chunk (one DMA each, pipelined); weights one DMA each
            x_sbs = [const.tile([P, ND, SC], BF, tag=f"x{c}", name=f"x{c}")
                     for c in range(NQ)]
            wq_sb = const.tile([P, ND, E], BF, tag="wq", name="wq")
            wk_sb = const.tile([P, ND, E], BF, tag="wk", name="wk")
            wv_sb = const.tile([P, ND, E], BF, tag="wv", name="wv")
            wo_sb = const.tile([P, NEB, D], BF, tag="wo", name="wo")
            bqk_sb = const.tile([P, NEB, 2], F32, tag="bqk", name="bqk")
            # Q^T/K^T per (e-block, q-chunk); V per 512-wide k-chunk
            qts = [[const.tile([P, SC], BF, tag=f"qt{h}{c}", name=f"qt{h}{c}") for c in range(NQ)]
                   for h in range(HPC)]
            kts = [[const.tile([P, SC], BF, tag=f"kt{e}{c}", name=f"kt{e}{c}") for c in range(NQ)]
                   for e in range(NEB)]
            v_sbs = [const.tile([P, NQ, HPC * (DH + 1)], BF, tag=f"v{i}", name=f"v{i}")
                     for i in range(NQ)]
            at_sbs = [[const.tile([P, SC], BF, tag=f"at{i}{f}", name=f"at{i}{f}")
                       for f in range(NEB)] for i in range(NQ)]
            mk_sb = const.tile([P, NQ, SC], BF, tag="mk", name="mk")
            # packed mask for the fused (m=1, m=3) diagonal pair: columns
            # 0:384 hold m1's active window (== m0's mask restricted there),
            # columns 384:512 hold m3's
            mkp_sb = const.tile([P, SC], BF, tag="mkp", name="mkp")
            # sel[hh][0, m] = 1 iff head hh of the pair owns at-row m (K=1
            # outer products broadcast both heads' softmax reciprocals into
            # one [128, SC] tile)
            # sel2[hh]: row 0 = head hh's partition-range selector, rows
            # 1..127 zero; rc2 row 0 = reciprocal row, rows 1..127 zero =>
            # the broadcast matmul contracts K=128 at full rate
            sel_sbs = [const.tile([P, P], BF, tag=f"sel{hh}", name=f"sel{hh}")
                       for hh in range(2)]
            rc2 = const.tile([P, 2, SC], BF, tag="rc2", name="rc2")
            nc.vector.memset(rc2[:], 0.0)

            # ---- input DMAs: one per tensor / half-x-chunk, compute-ordered
            xTr = xT.rearrange("(n p) s -> p n s", p=P)

            def dma_x(c, part, of):
                nd = ND // of
                nc.sync.dma_start(
                    x_sbs[c][:, part * nd:(part + 1) * nd, :],
                    xTr[:, part * nd:(part + 1) * nd, c * SC:(c + 1) * SC])
            if ablate != "nodma":
                wkTr = wkT.rearrange("(n p) e -> p n e", p=P)
                nd2 = ND // 2
                # x chunk 0 in quarters interleaved with wk halves so the
                # first K-projection matmuls unblock as early as possible
                nc.sync.dma_start(wk_sb[:, :nd2, :], wkTr[:, :nd2, :])
                dma_x(0, 0, 4)
                dma_x(0, 1, 4)
                nc.sync.dma_start(wk_sb[:, nd2:, :], wkTr[:, nd2:, :])
                dma_x(0, 2, 4)
                dma_x(0, 3, 4)
                nc.sync.dma_start(bqk_sb[:], bqk.rearrange("(n p) two -> p n two", p=P))
                nc.sync.dma_start(wq_sb[:], wqT.rearrange("(n p) e -> p n e", p=P))
                nc.sync.dma_start(wv_sb[:], wvT.rearrange("(n p) e -> p n e", p=P))
                for c in range(1, NQ):
                    dma_x(c, 0, 2)
                    dma_x(c, 1, 2)
                nc.sync.dma_start(wo_sb[:], woT.rearrange("(n p) d -> p n d", p=P))
            else:
                nc.vector.memset(bqk_sb[:], 0.0)

            # constants: head-pair selector + multiplicative causal masks
            tmp = ctx.enter_context(tc.tile_pool(name="tmp", bufs=1))
            one_f32 = tmp.tile([P, 1], F32, tag="onef", name="onef")
            nc.vector.memset(one_f32[:], 1.0)
            for hh in range(2):
                nc.vector.memset(sel_sbs[hh][:], 0.0)
                nc.vector.memset(sel_sbs[hh][0:1, hh * DH:(hh + 1) * DH], 1.0)
            # mk[m][kk, qq] = 1.0 if kk + 128*m <= qq else 0.0
            mkf_sb = tmp.tile([P, NQ, SC], F32, tag="mkf", name="mkf")
            for m in range(NQ):
                nc.gpsimd.memset(mkf_sb[:, m, :], 1.0)
                nc.gpsimd.affine_select(
                    out=mkf_sb[:, m, :], in_=mkf_sb[:, m, :],
                    compare_op=mybir.AluOpType.is_ge, fill=0.0,
                    base=-(P * m), pattern=[[1, SC]], channel_multiplier=-1,
                )
            nc.vector.tensor_copy(mk_sb[:], mkf_sb[:])
            nc.vector.tensor_copy(mkp_sb[:, 0:384], mkf_sb[:, 0, 0:384])
            nc.vector.tensor_copy(mkp_sb[:, 384:], mkf_sb[:, 3, 384:])
            for cc in range(NQ):
                nc.vector.tensor_copy(
                    v_sbs[cc][:, :, DH::DH + 1],
                    one_f32[:, :, None].broadcast_to([P, NQ, HPC]))
            for h in range(HPC):
                hh = h % 2
                for cc in range(NQ):
                    nc.gpsimd.memset(qts[h][cc][(1 - hh) * DH:(2 - hh) * DH, :], 0.0)

            # PSUM: pproj 1 + psc 4 + pav 2 + pmx 1 = 8 banks.
            pproj = ctx.enter_context(tc.tile_pool(name="pproj", bufs=1, space="PSUM"))
            psc = ctx.enter_context(tc.tile_pool(name="psc", bufs=2, space="PSUM"))
            pav = ctx.enter_context(tc.tile_pool(name="pav", bufs=1, space="PSUM"))
            pmx = ctx.enter_context(tc.tile_pool(name="pmx", bufs=1, space="PSUM"))
            ptp = ctx.enter_context(tc.tile_pool(name="ptp", bufs=min(2 * lag + 6, 28)))
            rcp = ctx.enter_context(tc.tile_pool(name="rcp", bufs=2))
            obp = ctx.enter_context(tc.tile_pool(name="obp", bufs=2))

            # ---- work-item generators (each item ~1 PSUM group on PE) ----
            def proj_pool(pool):
                # allocate a [P, SC] f32 group tile from the given PSUM pool
                # (tag must match the pool's existing tag to avoid growing it)
                tag = {id(pproj): "pj", id(pmx): "mx", id(psc): "sc"}[id(pool)]
                return pool.tile([P, SC], F32, tag=tag, name="pjx")

            def proj_qk_group(w_sb, bcol, o_tiles, c, eb, pool=None):
                ps = proj_pool(pool or pproj)
                for di in range(ND):
                    nc.tensor.matmul(
                        ps[:],
                        lhsT=w_sb[:, di, eb * P:(eb + 1) * P],
                        rhs=x_sbs[c][:, di, :],
                        start=(di == 0), stop=(di == ND - 1),
                    )
                if bcol == 1:   # K^T: packed two-head tile
                    nc.vector.tensor_scalar_add(
                        out=o_tiles[eb][c][:], in0=ps[:],
                        scalar1=bqk_sb[:, eb, bcol:bcol + 1])
                else:           # Q^T: per-head zero-padded tiles
                    for hh in range(2):
                        rows = slice(hh * DH, (hh + 1) * DH)
                        nc.vector.tensor_scalar_add(
                            out=o_tiles[2 * eb + hh][c][rows, :],
                            in0=ps[rows, :],
                            scalar1=bqk_sb[rows, eb, 0:1])

            def proj_v_group(c, kk, pool=None):
                ps = proj_pool(pool or pproj)
                for di in range(ND):
                    nc.tensor.matmul(
                        ps[:, :E],
                        lhsT=x_sbs[c][:, di, kk * P:(kk + 1) * P],
                        rhs=wv_sb[:, di, :],
                        start=(di == 0), stop=(di == ND - 1),
                    )
                dst = v_sbs[c][:, kk, :].rearrange(
                    "p (h e) -> p h e", h=HPC)[:, :, :DH]
                nc.vector.tensor_copy(
                    dst, ps[:, :E].rearrange("p (h e) -> p h e", h=HPC))

            def proj_items(c, pools=None):
                # pools: optional PSUM rotation for the startup prologue,
                # when the attention pools are still idle
                pl = (lambda i: pools[i % len(pools)]) if pools else \
                    (lambda i: None)
                items = []
                for eb in range(NEB):
                    items.append(lambda eb=eb, i=len(items):
                                 proj_qk_group(wk_sb, 1, kts, c, eb, pl(i)))
                for eb in range(NEB):
                    items.append(lambda eb=eb, i=len(items):
                                 proj_qk_group(wq_sb, 0, qts, c, eb, pl(i)))
                for kk in range(NQ):
                    items.append(lambda kk=kk, i=len(items):
                                 proj_v_group(c, kk, pl(i)))
                return items

            outTr = outT.rearrange("(n p) s -> p n s", p=P)
            ob_tiles = {}

            def outproj_group(c, eb, pool=None):
                if eb == 0:
                    ob_tiles[c] = obp.tile([P, ND, SC], BF, tag="ob", name="ob")
                po = proj_pool(pool or pmx)
                for ft in range(NEB):
                    nc.tensor.matmul(
                        po[:],
                        lhsT=wo_sb[:, ft, eb * P:(eb + 1) * P],
                        rhs=at_sbs[c][ft][:],
                        start=(ft == 0), stop=(ft == NEB - 1),
                    )
                tail = c == NQ - 1
                if tail and eb == ND - 2:
                    # second-to-last eviction on ACT (idle at the tail;
                    # Copy shares Exp's activation table) so the last two
                    # evictions run on different engines in parallel
                    nc.scalar.activation(ob_tiles[c][:, eb, :], po[:],
                                         AF.Copy)
                else:
                    nc.vector.tensor_copy(ob_tiles[c][:, eb, :], po[:])
                # store in halves; quarters for the last chunk (singles for
                # its final two blocks) so the DMA on the kernel critical
                # tail is as small as possible
                grp = 1 if (tail and eb >= ND - 2) else \
                    (2 if tail else ND // 2)
                if (eb + 1) % grp == 0:
                    lo = eb + 1 - grp
                    nc.sync.dma_start(
                        outTr[:, lo:eb + 1, c * SC:(c + 1) * SC],
                        ob_tiles[c][:, lo:eb + 1, :])

            def outproj_items(c, pools=(None,)):
                # pools: PSUM bank rotation so group k+1's matmuls don't
                # wait on group k's eviction (pass idle pools only)
                return [lambda eb=eb: outproj_group(c, eb,
                                                    pools[eb % len(pools)])
                        for eb in range(ND)]

            # ---- attention for chunk c, with fill items interleaved ----
            # Heads run in pairs (one pass over all k-blocks per pair): the
            # pair's two score matmuls land in one 2-bank PSUM tile so a
            # single wide Exp covers both heads (halves ACT instruction
            # overhead), and only 2 AV accumulator banks are live at a time.
            pend = []

            def flush_one():
                c0, hp, item, pt, first, last, av_fns = pend.pop(0)
                av_fns(c0, hp, item, pt, first, last)

            def attention_chunk(c, fill):
                nj = NQ * (c + 1)
                filled = 0

                # Work items: plain k-blocks, then the diagonal emitted
                # as m0, m2, and a fused (m1, m3) pair that shares one
                # PSUM bank / Exp / mask per head (the pair's 384+128
                # active columns exactly fill a bank). Each item is a
                # list of (j, q0, col0) score segments.
                items = [[(j, 0, 0)] for j in range(NQ * c)]
                items.append([(NQ * c, 0, 0)])                    # m0
                items.append([(NQ * c + 2, 2 * P, 2 * P)])        # m2
                items.append([(NQ * c + 1, P, 0),                 # m1
                              (NQ * c + 3, 3 * P, 3 * P)])        # m3
                ni = len(items)
                nsteps = 2 * ni

                def bankcols(item, q0, col0):
                    # bank columns backing q-range [q0:SC] of a segment:
                    # packed segments sit at [col0 : col0 + width]
                    if len(item) > 1:
                        return slice(col0, col0 + (SC - q0))
                    return slice(q0, SC)

                def banklo(item):
                    return 0 if len(item) > 1 else item[0][1]

                def scores(hp, item):
                    # both heads' scores land in one 2-bank psum tile so a
                    # single wide Exp covers the pair. Contraction is K=128:
                    # the packed K^T e-block against the per-head zero-padded
                    # Q^T (K<=64 matmuls cost ~2x cycles/col on HW).
                    pt = ptp.tile([P, 2, SC], BF, tag="pt", name="pt")
                    packed = len(item) > 1
                    lo = banklo(item)
                    ps = psc.tile([P, 2, SC], F32, tag="sc", name="sc")
                    for j, q0, col0 in item:
                        for hh in range(2):  # hh inner: shared kts stationary
                            nc.tensor.matmul(
                                ps[:, hh, bankcols(item, q0, col0)],
                                lhsT=kts[hp][j // NQ][:, (j % NQ) * P:(j % NQ + 1) * P],
                                rhs=qts[2 * hp + hh][c][:, q0:],
                                start=True, stop=True,
                            )
                    nc.scalar.activation(pt[:, :, lo:], ps[:, :, lo:],
                                         AF.Exp, scale=SCALE)
                    m = item[0][0] - NQ * c
                    if m >= 0:   # diagonal: mask (packed pair has mkp)
                        mk = mkp_sb[:, lo:] if packed else mk_sb[:, m, lo:]
                        nc.gpsimd.tensor_mul(
                            pt[:, :, lo:], pt[:, :, lo:],
                            mk[:, None, :].broadcast_to([P, 2, SC - lo]))
                    return pt

                av_tiles = {}

                def avs(hp, item, pt, first, last):
                    if first:
                        av_tiles[hp] = [pav.tile([DH + 1, SC], F32,
                                                 tag=f"av{i}", name=f"av{i}")
                                        for i in range(2)]
                    for hh in range(2):
                        h = 2 * hp + hh
                        for idx, (j, q0, col0) in enumerate(item):
                            nc.tensor.matmul(
                                av_tiles[hp][hh][:, q0:],
                                lhsT=v_sbs[j // NQ][:, j % NQ,
                                                    h * (DH + 1):(h + 1) * (DH + 1)],
                                rhs=pt[:, hh, bankcols(item, q0, col0)],
                                start=first and idx == 0,
                                stop=last and idx == len(item) - 1,
                            )
                    if last:
                        normalize(hp)

                def normalize(hp):
                    # A^T[f, q] = av[f, q] * (1 / denom[q]); both heads'
                    # reciprocal rows are broadcast into one [128, SC] tile
                    # via two accumulating K=1 outer products with the
                    # selector rows (DVE writes must start at partition 0,
                    # so the reciprocals live in separate single-row tiles).
                    rb_ps = pmx.tile([P, SC], F32, tag="mx", name="mx")
                    with nc.allow_low_precision(
                            reason="bf16 rounding of softmax recip is benign"):
                        for hh in range(2):
                            nc.vector.reciprocal(
                                rc2[0:1, hh, :], av_tiles[hp][hh][DH:DH + 1, :])
                            nc.tensor.matmul(
                                rb_ps[:], lhsT=sel_sbs[hh][:, :],
                                rhs=rc2[:, hh, :],
                                start=(hh == 0), stop=(hh == 1))
                    rcb = rcp.tile([P, SC], BF, tag="rcb", name="rcb")
                    nc.vector.tensor_copy(rcb[:], rb_ps[:])
                    with nc.allow_low_precision(
                            reason="bf16 attention weights are benign"):
                        for hh in range(2):
                            nc.vector.tensor_mul(
                                at_sbs[c][hp][hh * DH:(hh + 1) * DH, :],
                                av_tiles[hp][hh][0:DH, :],
                                rcb[hh * DH:(hh + 1) * DH, :])

                # single merged stream over both head-pair passes AND across
                # chunks: the pend queue carries over so chunk c+1's scores
                # overlap chunk c's AV drain / normalize (no boundary bubble)
                def av_fns(c0, hp, item, pt, first, last):
                    avs(hp, item, pt, first, last)

                stream = [(hp, i) for hp in range(2) for i in range(ni)]
                for step, (hp, i) in enumerate(stream):
                    pend.append((c, hp, items[i], scores(hp, items[i]),
                                 i == 0, i == ni - 1, av_fns))
                    if len(pend) > lag:
                        flush_one()
                    want = (step + 1) * len(fill) // nsteps
                    while filled < want:
                        fill[filled]()
                        filled += 1

            # ---- software-pipelined schedule ----
            # proj(0) runs standalone (it is the DMA-paced startup); then
            # attention(c) hides proj(c+1) and outproj(c-1); outproj(3) tails.
            for item in proj_items(0, pools=[pproj, pmx, psc, psc]):
                item()
            rot2 = (pmx, pproj)
            fills = {0: proj_items(1),
                     1: proj_items(2),
                     2: proj_items(3) + outproj_items(0),
                     3: outproj_items(1, rot2) + outproj_items(2, rot2)}
            for c in range(NQ):
                attention_chunk(c, fills[c])
            while pend:
                flush_one()
            # pure tail: psc's banks are free too — 3-deep rotation hides
            # the PSUM-eviction latency between groups
            for item in outproj_items(NQ - 1, (pmx, pproj, psc)):
                item()

    _split_multiwait(nc)
    return nc


_NC_CACHE = None


def _shard_inputs(inputs):
    bf = ml_dtypes.bfloat16
    x = np.asarray(inputs["x"], np.float32)
    Wq = np.asarray(inputs["Wq"], np.float32)
    Wk = np.asarray(inputs["Wk"], np.float32)
    Wv = np.asarray(inputs["Wv"], np.float32)
    Wo = np.asarray(inputs["Wo"], np.float32)
    bq = np.asarray(inputs["bq"], np.float32)
    bk = np.asarray(inputs["bk"], np.float32)

    xTs = [np.ascontiguousarray(x[b].T).astype(bf) for b in range(B)]
    in_maps = []
    for c in range(N_CORES):
        b, g = divmod(c, GROUPS)
        rows = slice(g * E, (g + 1) * E)
        in_maps.append({
            "xT": xTs[b],
            "wqT": np.ascontiguousarray(Wq[rows].T).astype(bf),
            "wkT": np.ascontiguousarray(Wk[rows].T).astype(bf),
            "wvT": np.ascontiguousarray(Wv[rows].T).astype(bf),
            "woT": np.ascontiguousarray(Wo[:, rows].T).astype(bf),
            "bqk": np.ascontiguousarray(
                np.stack([bq[rows], bk[rows]], axis=1)),
        })
    return in_maps


def kernel(**inputs):
    global _NC_CACHE
    if _NC_CACHE is None:
        _NC_CACHE = build_nc()
    nc = _NC_CACHE

    # The mask input is causal (tril ones) by construction; the kernel
    # hardcodes causal structure.
    in_maps = _shard_inputs(inputs)
    res = run_bass_kernel_spmd(nc, in_maps, list(range(N_CORES)))

    Wo = np.asarray(inputs["Wo"], np.float32)
    bv = np.asarray(inputs["bv"], np.float32)
    bo = np.asarray(inputs["bo"], np.float32)
    out = np.zeros((B, S, D), np.float32)
    for c in range(N_CORES):
        b = c // GROUPS
        out[b] += res.results[c]["outT"].astype(np.float32).T
    # bv enters only additively after softmax (rows of P sum to 1):
    # out += Wo @ bv; plus the output bias bo.
    out += (Wo @ bv + bo)[None, None, :]
    return out

